# revision 32
# baseline (speedup 1.0000x reference)
"""Trainium2 Bass kernel for nn_LinearLatentKernel_84834194031187.

Computes, for x:[B,S,D], W_qkv:[3D,D], W_gate:[D,D] (fp32):
    qkv = x @ W_qkv.T + b_qkv ; q,k,v = split(qkv)
    kv_state = cumsum(k*v, axis=seq)
    out = q * kv_state * sigmoid(x @ W_gate.T + b_gate)

Sharding: 8-way channel split. Core h handles channels [h*128,(h+1)*128) of
q, k, v and the gate for ALL four batches, producing out[:, :, h*128:...].
This keeps each core's weight slice at 1MB (vs 4MB for a batch x half-D
split), which matters because the startup ramp is HBM-bandwidth-bound:
block 0 cannot finish before its weights land.

x is host-pretransposed and pre-tiled into [NBLK, 128, B, KT, 128] fp16 so
each seq block's x^T tiles (all 4 batches) arrive via contiguous DMAs
(8KB/partition) and feed the PE stationary port directly -- no on-device
transposes.

Per seq block of 128 rows (partition dim = seq):
  - one PSUM bank per batch accumulates [k|v|q|g] x 128 channels over the
    8 contraction tiles (fp16 operands, fp32 PSUM, N=512 matmuls). The
    batch-outer matmul order staggers bank completion so each bank is
    drained (kv product, sigmoid, q*g on DVE/ACT) while later batches'
    matmuls still run: all four banks stay single-buffered with no stalls.
  - carry fold: kv[0,:] += carry (one DVE add covering all batches); a
    single upper-triangular fp16 matmul (N=512: the four batches' columns
    are independent) then yields the block cumsum INCLUDING the carry, and
    its row 127 IS the carry for the next block: a 1-lane scalar copy +
    SBUF-to-SBUF DMA moves it from partition 127 to 0.
  - The PE queue is software-pipelined one block: cumsum matmuls for block
    i-1 are enqueued between batch groups of block i's projections.
  - out = (q * sigmoid(g)) * kv_state, one 256KB DMA per block.

A short burst of dummy "warmup" matmuls (no DMA dependencies) runs first so
the PE's HAM reaches its fast state during the initial DMA ramp.
"""

import numpy as np

import concourse.bacc as bacc
import concourse.tile as tile
import concourse.mybir as mybir
from concourse.bass_utils import run_bass_kernel_spmd

B, S, D = 4, 4096, 1024
HC = 128         # channels per core (D / 8 cores)
W4 = 4 * HC      # k|v|q|g channel block per core = 512
P = 128
NBLK = S // P    # 32 seq blocks
KT = D // P      # 8 contraction tiles

f32 = mybir.dt.float32
f16 = mybir.dt.float16

_NC_CACHE = {}


def _build(with_bias: bool):
    nc = bacc.Bacc("TRN2", target_bir_lowering=False)

    # x^T pre-tiled on host: xh[i, p, b, kt, j] = x[b, i*128+j, kt*128+p]
    xh_d = nc.dram_tensor("xh", [NBLK, P, B, KT, P], f16, kind="ExternalInput")
    # weight columns ordered [k | v | q | g], HC channels each
    wt_d = nc.dram_tensor("wt", [KT, P, W4], f16, kind="ExternalInput")
    tri_d = nc.dram_tensor("tri", [P, P], f16, kind="ExternalInput")
    if with_bias:
        onesrow_d = nc.dram_tensor("onesrow", [1, P], f16, kind="ExternalInput")
        bias_d = nc.dram_tensor("bias", [1, W4], f16, kind="ExternalInput")
    # out[i, j, b, c] = result[b, i*128+j, h*128+c]
    out_d = nc.dram_tensor("out", [NBLK, P, B, HC], f32, kind="ExternalOutput")

    with tile.TileContext(nc) as tc:
        with (
            tc.tile_pool(name="consts", bufs=1) as consts,
            tc.tile_pool(name="xtp", bufs=3) as xtp,
            tc.tile_pool(name="kp", bufs=2) as kp,
            tc.tile_pool(name="gp", bufs=2) as gp,
            tc.tile_pool(name="kvp", bufs=2) as kvp,
            tc.tile_pool(name="qgp", bufs=2) as qgp,
            tc.tile_pool(name="outp", bufs=3) as outp,
            tc.tile_pool(name="tmpp", bufs=2) as tmpp,
            tc.tile_pool(name="carryp", bufs=2) as carryp,
            tc.tile_pool(name="pmm", bufs=1, space="PSUM") as pmm,
            tc.tile_pool(name="pcs_pool", bufs=2, space="PSUM") as pcs_pool,
            tc.tile_pool(name="pwm", bufs=1, space="PSUM") as pwm,
        ):
            warm_a = consts.tile([P, P], f16, tag="warm_a")
            nc.vector.memset(warm_a[:], 0.0)
            warm_b = consts.tile([P, 512], f16, tag="warm_b")
            nc.vector.memset(warm_b[:], 0.0)
            pwarm = pwm.tile([P, 512], f32, tag="pwarm")

            def warm(n):
                for _ in range(n):
                    nc.tensor.matmul(pwarm[:], warm_a[:], warm_b[:],
                                     start=True, stop=True)

            warm(12)

            # block 0/1 inputs split across two trigger queues; weights on
            # gpsimd+scalar so everything rides different DMA rings
            xt0 = xtp.tile([P, B, KT, P], f16, tag="xt", name="xt0")
            nc.sync.dma_start(xt0[:, 0:2], xh_d[0, :, 0:2])
            nc.scalar.dma_start(xt0[:, 2:4], xh_d[0, :, 2:4])
            wt_sb = consts.tile([P, KT, W4], f16, tag="wt")
            for kt in range(KT):
                eng = nc.gpsimd if kt % 2 == 0 else nc.scalar
                eng.dma_start(wt_sb[:, kt, :], wt_d[kt])
            xt1 = xtp.tile([P, B, KT, P], f16, tag="xt", name="xt1")
            nc.sync.dma_start(xt1[:, 0:2], xh_d[1, :, 0:2])
            nc.scalar.dma_start(xt1[:, 2:4], xh_d[1, :, 2:4])

            tri_sb = consts.tile([P, P], f16, tag="tri")
            nc.sync.dma_start(tri_sb[:], tri_d[:])
            if with_bias:
                onesrow_sb = consts.tile([1, P], f16, tag="onesrow")
                nc.sync.dma_start(onesrow_sb[:], onesrow_d[:])
                bias_sb = consts.tile([1, W4], f16, tag="bias")
                nc.sync.dma_start(bias_sb[:], bias_d[:])

            xts = {0: xt0, 1: xt1}
            pending = None      # (kvs, qgs, i) awaiting cumsum+output

            def proj_batch(ps_b, xt, b):
                for kt in range(KT):
                    nc.tensor.matmul(
                        ps_b[:], xt[:, b, kt, :], wt_sb[:, kt, :],
                        start=(kt == 0),
                        stop=(kt == KT - 1 and not with_bias),
                    )
                if with_bias:
                    nc.tensor.matmul(ps_b[:], onesrow_sb[:], bias_sb[:],
                                     start=False, stop=True)

            def cumsum_mms(pend):
                # PE part of block j's cumsum: ONE N=512 tri matmul covers all
                # four batches (columns independent); row 127 = next carry,
                # moved 127->0 via 1-lane copy + tiny SBUF DMA.
                kv_all, qg_all, j = pend
                pcs = pcs_pool.tile([P, B, HC], f32, tag="pcs")
                nc.tensor.matmul(pcs[:], tri_sb[:], kv_all[:],
                                 start=True, stop=True)
                carry_new = None
                if j < NBLK - 1:
                    tmp = tmpp.tile([P, B, HC], f32, tag="tmp")
                    nc.scalar.activation(tmp[96:P], pcs[96:P],
                                         mybir.ActivationFunctionType.Copy)
                    carry_new = carryp.tile([1, B, HC], f32, tag="carry")
                    nc.gpsimd.dma_start(carry_new[0:1], tmp[P - 1:P])
                return pcs, carry_new

            def emit_out(pend, pcs):
                _, qg_all, j = pend
                ob = outp.tile([P, B, HC], f32, tag="ob")
                nc.vector.tensor_mul(out=ob[:], in0=qg_all[:], in1=pcs[:])
                nc.sync.dma_start(out_d[j], ob[:])

            for i in range(NBLK):
                if i + 2 < NBLK:
                    xt = xtp.tile([P, B, KT, P], f16, tag="xt")
                    nc.sync.dma_start(xt[:, 0:2], xh_d[i + 2, :, 0:2])
                    nc.scalar.dma_start(xt[:, 2:4], xh_d[i + 2, :, 2:4])
                    xts[i + 2] = xt
                xt = xts.pop(i)

                ps = [pmm.tile([P, W4], f32, tag=f"ps{b}", name=f"ps{b}")
                      for b in range(B)]
                kv_all = kvp.tile([P, B, HC], f16, tag="kv")
                qg_all = qgp.tile([P, B, HC], f32, tag="qg")

                proj_batch(ps[0], xt, 0)
                proj_batch(ps[1], xt, 1)

                # block i-1's cumsum matmuls, mid-block on the PE queue
                pcs_prev = None
                if pending is not None:
                    pcs_prev, carry_prev = cumsum_mms(pending)

                proj_batch(ps[2], xt, 2)
                proj_batch(ps[3], xt, 3)

                for b in range(B):
                    k_sb = kp.tile([P, HC], f32, tag=f"k{b}")
                    nc.scalar.activation(k_sb[:], ps[b][:, 0:HC],
                                         mybir.ActivationFunctionType.Copy)
                    nc.vector.tensor_mul(out=kv_all[:, b, :], in0=k_sb[:],
                                         in1=ps[b][:, HC:2 * HC])
                    if b == 0 and pending is not None:
                        emit_out(pending, pcs_prev)
                    g_sb = gp.tile([P, HC], f32, tag=f"g{b}")
                    nc.scalar.activation(g_sb[:], ps[b][:, 3 * HC:4 * HC],
                                         mybir.ActivationFunctionType.Sigmoid)
                    nc.vector.tensor_mul(out=qg_all[:, b, :], in0=g_sb[:],
                                         in1=ps[b][:, 2 * HC:3 * HC])
                    if i > 0:
                        # carry fold: kv[0,:] += carry (cumsum row 127);
                        # per batch so each only depends on its kv product
                        nc.vector.tensor_add(out=kv_all[0:1, b, :],
                                             in0=kv_all[0:1, b, :],
                                             in1=carry_prev[0:1, b, :])

                pending = (kv_all, qg_all, i)

            # final flush: per-batch cumsum + product + store, so each
            # piece drains as soon as its own kv/fold is ready
            kv_all, qg_all, j = pending
            pcs = pcs_pool.tile([P, B, HC], f32, tag="pcs")
            ob = outp.tile([P, B, HC], f32, tag="ob")
            for b in range(B):
                nc.tensor.matmul(pcs[:, b, :], tri_sb[:], kv_all[:, b, :],
                                 start=True, stop=True)
                nc.vector.tensor_mul(out=ob[:, b, :], in0=qg_all[:, b, :],
                                     in1=pcs[:, b, :])
                nc.sync.dma_start(out_d[j, :, b], ob[:, b, :])

    nc.compile()
    return nc


def _get_nc(with_bias: bool):
    if with_bias not in _NC_CACHE:
        _NC_CACHE[with_bias] = _build(with_bias)
    return _NC_CACHE[with_bias]


def _prep_in_maps(x, W_qkv, b_qkv, W_gate, b_gate, with_bias):
    x = np.asarray(x, dtype=np.float32).astype(np.float16)
    W_qkv = np.asarray(W_qkv, dtype=np.float32)
    W_gate = np.asarray(W_gate, dtype=np.float32)

    consts = {
        "tri": np.triu(np.ones((P, P), dtype=np.float16)),
    }
    if with_bias:
        consts["onesrow"] = np.ones((1, P), dtype=np.float16)

    # xh[i, p, b, kt, j] = x[b, i*128+j, kt*128+p]  (shared by all cores)
    xh = np.ascontiguousarray(
        x.reshape(B, NBLK, P, KT, P).transpose(1, 4, 0, 3, 2))

    in_maps = []
    for h in range(8):
        sl = slice(h * HC, (h + 1) * HC)
        wt = np.concatenate(
            [W_qkv[D + h * HC:D + (h + 1) * HC],        # k rows
             W_qkv[2 * D + h * HC:2 * D + (h + 1) * HC],  # v rows
             W_qkv[sl],                                   # q rows
             W_gate[sl]], axis=0                          # g rows
        ).T.astype(np.float16)                            # [D, 512]
        wt = np.ascontiguousarray(wt.reshape(KT, P, W4))
        m = {"xh": xh, "wt": wt, **consts}
        if with_bias:
            bq = np.asarray(b_qkv, dtype=np.float32)
            bg = np.asarray(b_gate, dtype=np.float32)
            m["bias"] = np.concatenate(
                [bq[D + h * HC:D + (h + 1) * HC],
                 bq[2 * D + h * HC:2 * D + (h + 1) * HC],
                 bq[sl], bg[sl]]
            )[None, :].astype(np.float16).copy()
        in_maps.append(m)
    return in_maps


def run(x, W_qkv, b_qkv, W_gate, b_gate, trace=False, **run_kwargs):
    with_bias = bool(np.any(np.asarray(b_qkv)) or np.any(np.asarray(b_gate)))
    nc = _get_nc(with_bias)
    in_maps = _prep_in_maps(x, W_qkv, b_qkv, W_gate, b_gate, with_bias)
    res = run_bass_kernel_spmd(nc, in_maps, list(range(8)), trace=trace, **run_kwargs)
    out = np.empty((B, S, D), dtype=np.float32)
    for h in range(8):
        # res[h]["out"]: [NBLK, P, B, HC] -> out[b, s, h*HC:(h+1)*HC]
        o = np.asarray(res.results[h]["out"]).transpose(2, 0, 1, 3)
        out[:, :, h * HC:(h + 1) * HC] = o.reshape(B, S, HC)
    return out, res


def kernel(x, W_qkv, b_qkv, W_gate, b_gate):
    out, _ = run(x, W_qkv, b_qkv, W_gate, b_gate)
    return out


# revision 33
# speedup vs baseline: 1.1946x; 1.1946x over previous
"""Trainium2 Bass kernel for nn_LinearLatentKernel_84834194031187.

Computes, for x:[B,S,D], W_qkv:[3D,D], W_gate:[D,D] (fp32):
    qkv = x @ W_qkv.T + b_qkv ; q,k,v = split(qkv)
    kv_state = cumsum(k*v, axis=seq)
    out = q * kv_state * sigmoid(x @ W_gate.T + b_gate)

Sharding: 8-way channel split. Core h handles channels [h*128,(h+1)*128) of
q, k, v and the gate for ALL four batches, producing out[:, :, h*128:...].
This keeps each core's weight slice at 1MB (vs 4MB for a batch x half-D
split), which matters because the startup ramp is HBM-bandwidth-bound:
block 0 cannot finish before its weights land.

x is host-pretransposed and pre-tiled into [NBLK, 128, B, KT, 128] fp16 so
each seq block's x^T tiles (all 4 batches) arrive via contiguous DMAs
(8KB/partition) and feed the PE stationary port directly -- no on-device
transposes.

Per seq block of 128 rows (partition dim = seq):
  - one PSUM bank per batch accumulates [k|v|q|g] x 128 channels over the
    8 contraction tiles (fp16 operands, fp32 PSUM, N=512 matmuls). The
    batch-outer matmul order staggers bank completion so each bank is
    drained (kv product, sigmoid, q*g on DVE/ACT) while later batches'
    matmuls still run: all four banks stay single-buffered with no stalls.
  - carry fold: kv[0,:] += carry (one DVE add covering all batches); a
    single upper-triangular fp16 matmul (N=512: the four batches' columns
    are independent) then yields the block cumsum INCLUDING the carry, and
    its row 127 IS the carry for the next block: a 1-lane scalar copy +
    SBUF-to-SBUF DMA moves it from partition 127 to 0.
  - The PE queue is software-pipelined one block: cumsum matmuls for block
    i-1 are enqueued between batch groups of block i's projections.
  - out = (q * sigmoid(g)) * kv_state, one 256KB DMA per block.

A short burst of dummy "warmup" matmuls (no DMA dependencies) runs first so
the PE's HAM reaches its fast state during the initial DMA ramp.
"""

import numpy as np

import concourse.bacc as bacc
import concourse.tile as tile
import concourse.mybir as mybir
from concourse.bass_utils import run_bass_kernel_spmd

B, S, D = 4, 4096, 1024
HC = 128         # channels per core (D / 8 cores)
W4 = 4 * HC      # k|v|q|g channel block per core = 512
P = 128
NBLK = S // P    # 32 seq blocks
KT = D // P      # 8 contraction tiles

f32 = mybir.dt.float32
f16 = mybir.dt.float16

_NC_CACHE = {}


def _build(with_bias: bool):
    nc = bacc.Bacc("TRN2", target_bir_lowering=False)

    # x^T pre-tiled on host: xh[i, p, b, kt, j] = x[b, i*128+j, kt*128+p]
    xh_d = nc.dram_tensor("xh", [NBLK, P, B, KT, P], f16, kind="ExternalInput")
    # weight columns ordered [k | v | q | g], HC channels each
    wt_d = nc.dram_tensor("wt", [KT, P, W4], f16, kind="ExternalInput")
    tri_d = nc.dram_tensor("tri", [P, P], f16, kind="ExternalInput")
    if with_bias:
        onesrow_d = nc.dram_tensor("onesrow", [1, P], f16, kind="ExternalInput")
        bias_d = nc.dram_tensor("bias", [1, W4], f16, kind="ExternalInput")
    # out[i, j, b, c] = result[b, i*128+j, h*128+c]
    out_d = nc.dram_tensor("out", [NBLK, P, B, HC], f32, kind="ExternalOutput")

    with tile.TileContext(nc) as tc:
        with (
            tc.tile_pool(name="consts", bufs=1) as consts,
            tc.tile_pool(name="xtp", bufs=3) as xtp,
            tc.tile_pool(name="kp", bufs=2) as kp,
            tc.tile_pool(name="gp", bufs=2) as gp,
            tc.tile_pool(name="kvp", bufs=2) as kvp,
            tc.tile_pool(name="qgp", bufs=2) as qgp,
            tc.tile_pool(name="outp", bufs=3) as outp,
            tc.tile_pool(name="tmpp", bufs=2) as tmpp,
            tc.tile_pool(name="carryp", bufs=2) as carryp,
            tc.tile_pool(name="pmm", bufs=1, space="PSUM") as pmm,
            tc.tile_pool(name="pcs_pool", bufs=2, space="PSUM") as pcs_pool,
            tc.tile_pool(name="pwm", bufs=1, space="PSUM") as pwm,
        ):
            warm_a = consts.tile([P, P], f16, tag="warm_a")
            nc.vector.memset(warm_a[:], 0.0)
            warm_b = consts.tile([P, 512], f16, tag="warm_b")
            nc.vector.memset(warm_b[:], 0.0)
            pwarm = pwm.tile([P, 512], f32, tag="pwarm")

            def warm(n):
                for _ in range(n):
                    nc.tensor.matmul(pwarm[:], warm_a[:], warm_b[:],
                                     start=True, stop=True)

            warm(12)

            # block 0/1 inputs split across two trigger queues; weights on
            # gpsimd+scalar so everything rides different DMA rings
            xt0 = xtp.tile([P, B, KT, P], f16, tag="xt", name="xt0")
            nc.sync.dma_start(xt0[:, 0:2], xh_d[0, :, 0:2])
            nc.scalar.dma_start(xt0[:, 2:4], xh_d[0, :, 2:4])
            wt_sb = consts.tile([P, KT, W4], f16, tag="wt")
            for kt in range(KT):
                eng = nc.gpsimd if kt % 2 == 0 else nc.scalar
                eng.dma_start(wt_sb[:, kt, :], wt_d[kt])
            xt1 = xtp.tile([P, B, KT, P], f16, tag="xt", name="xt1")
            nc.sync.dma_start(xt1[:, 0:2], xh_d[1, :, 0:2])
            nc.scalar.dma_start(xt1[:, 2:4], xh_d[1, :, 2:4])

            tri_sb = consts.tile([P, P], f16, tag="tri")
            nc.sync.dma_start(tri_sb[:], tri_d[:])
            if with_bias:
                onesrow_sb = consts.tile([1, P], f16, tag="onesrow")
                nc.sync.dma_start(onesrow_sb[:], onesrow_d[:])
                bias_sb = consts.tile([1, W4], f16, tag="bias")
                nc.sync.dma_start(bias_sb[:], bias_d[:])

            xts = {0: xt0, 1: xt1}
            pending = None      # (kvs, qgs, i) awaiting cumsum+output

            def proj_batch(ps_b, xt, b):
                for kt in range(KT):
                    nc.tensor.matmul(
                        ps_b[:], xt[:, b, kt, :], wt_sb[:, kt, :],
                        start=(kt == 0),
                        stop=(kt == KT - 1 and not with_bias),
                    )
                if with_bias:
                    nc.tensor.matmul(ps_b[:], onesrow_sb[:], bias_sb[:],
                                     start=False, stop=True)

            def cumsum_mms(pend):
                # PE part of block j's cumsum: ONE N=512 tri matmul covers all
                # four batches (columns independent); row 127 = next carry,
                # moved 127->0 via 1-lane copy + tiny SBUF DMA.
                kv_all, qg_all, j = pend
                pcs = pcs_pool.tile([P, B, HC], f32, tag="pcs")
                nc.tensor.matmul(pcs[:], tri_sb[:], kv_all[:],
                                 start=True, stop=True)
                carry_new = None
                if j < NBLK - 1:
                    tmp = tmpp.tile([P, B, HC], f32, tag="tmp")
                    nc.scalar.activation(tmp[96:P], pcs[96:P],
                                         mybir.ActivationFunctionType.Copy)
                    carry_new = carryp.tile([1, B, HC], f32, tag="carry")
                    nc.gpsimd.dma_start(carry_new[0:1], tmp[P - 1:P])
                return pcs, carry_new

            def emit_out(pend, pcs):
                _, qg_all, j = pend
                ob = outp.tile([P, B, HC], f32, tag="ob")
                nc.vector.tensor_mul(out=ob[:], in0=qg_all[:], in1=pcs[:])
                nc.sync.dma_start(out_d[j], ob[:])

            for i in range(NBLK):
                if i + 2 < NBLK:
                    xt = xtp.tile([P, B, KT, P], f16, tag="xt")
                    nc.sync.dma_start(xt[:, 0:2], xh_d[i + 2, :, 0:2])
                    nc.scalar.dma_start(xt[:, 2:4], xh_d[i + 2, :, 2:4])
                    xts[i + 2] = xt
                xt = xts.pop(i)

                ps = [pmm.tile([P, W4], f32, tag=f"ps{b}", name=f"ps{b}")
                      for b in range(B)]
                kv_all = kvp.tile([P, B, HC], f16, tag="kv")
                qg_all = qgp.tile([P, B, HC], f32, tag="qg")

                proj_batch(ps[0], xt, 0)
                proj_batch(ps[1], xt, 1)

                # block i-1's cumsum matmuls, mid-block on the PE queue
                pcs_prev = None
                if pending is not None:
                    pcs_prev, carry_prev = cumsum_mms(pending)

                proj_batch(ps[2], xt, 2)
                proj_batch(ps[3], xt, 3)

                for b in range(B):
                    k_sb = kp.tile([P, HC], f32, tag=f"k{b}")
                    nc.scalar.activation(k_sb[:], ps[b][:, 0:HC],
                                         mybir.ActivationFunctionType.Copy)
                    nc.vector.tensor_mul(out=kv_all[:, b, :], in0=k_sb[:],
                                         in1=ps[b][:, HC:2 * HC])
                    if i == NBLK - 1:
                        # final block: carry arrived long ago; folding right
                        # after the kv product unblocks the tail cumsum
                        nc.vector.tensor_add(out=kv_all[0:1, b, :],
                                             in0=kv_all[0:1, b, :],
                                             in1=carry_prev[0:1, b, :])
                    if b == 0 and pending is not None:
                        emit_out(pending, pcs_prev)
                    g_sb = gp.tile([P, HC], f32, tag=f"g{b}")
                    nc.scalar.activation(g_sb[:], ps[b][:, 3 * HC:4 * HC],
                                         mybir.ActivationFunctionType.Sigmoid)
                    nc.vector.tensor_mul(out=qg_all[:, b, :], in0=g_sb[:],
                                         in1=ps[b][:, 2 * HC:3 * HC])
                    if 0 < i < NBLK - 1:
                        # carry fold: kv[0,:] += carry (cumsum row 127);
                        # after qg so the PSUM drain never waits on the
                        # in-flight carry DMA
                        nc.vector.tensor_add(out=kv_all[0:1, b, :],
                                             in0=kv_all[0:1, b, :],
                                             in1=carry_prev[0:1, b, :])

                pending = (kv_all, qg_all, i)

            # final flush: per-batch cumsum + product + store, so each
            # piece drains as soon as its own kv/fold is ready
            kv_all, qg_all, j = pending
            pcs = pcs_pool.tile([P, B, HC], f32, tag="pcs")
            ob = outp.tile([P, B, HC], f32, tag="ob")
            for b in range(B):
                nc.tensor.matmul(pcs[:, b, :], tri_sb[:], kv_all[:, b, :],
                                 start=True, stop=True)
                nc.vector.tensor_mul(out=ob[:, b, :], in0=qg_all[:, b, :],
                                     in1=pcs[:, b, :])
                nc.sync.dma_start(out_d[j, :, b], ob[:, b, :])

    nc.compile()
    return nc


def _get_nc(with_bias: bool):
    if with_bias not in _NC_CACHE:
        _NC_CACHE[with_bias] = _build(with_bias)
    return _NC_CACHE[with_bias]


def _prep_in_maps(x, W_qkv, b_qkv, W_gate, b_gate, with_bias):
    x = np.asarray(x, dtype=np.float32).astype(np.float16)
    W_qkv = np.asarray(W_qkv, dtype=np.float32)
    W_gate = np.asarray(W_gate, dtype=np.float32)

    consts = {
        "tri": np.triu(np.ones((P, P), dtype=np.float16)),
    }
    if with_bias:
        consts["onesrow"] = np.ones((1, P), dtype=np.float16)

    # xh[i, p, b, kt, j] = x[b, i*128+j, kt*128+p]  (shared by all cores)
    xh = np.ascontiguousarray(
        x.reshape(B, NBLK, P, KT, P).transpose(1, 4, 0, 3, 2))

    in_maps = []
    for h in range(8):
        sl = slice(h * HC, (h + 1) * HC)
        wt = np.concatenate(
            [W_qkv[D + h * HC:D + (h + 1) * HC],        # k rows
             W_qkv[2 * D + h * HC:2 * D + (h + 1) * HC],  # v rows
             W_qkv[sl],                                   # q rows
             W_gate[sl]], axis=0                          # g rows
        ).T.astype(np.float16)                            # [D, 512]
        wt = np.ascontiguousarray(wt.reshape(KT, P, W4))
        m = {"xh": xh, "wt": wt, **consts}
        if with_bias:
            bq = np.asarray(b_qkv, dtype=np.float32)
            bg = np.asarray(b_gate, dtype=np.float32)
            m["bias"] = np.concatenate(
                [bq[D + h * HC:D + (h + 1) * HC],
                 bq[2 * D + h * HC:2 * D + (h + 1) * HC],
                 bq[sl], bg[sl]]
            )[None, :].astype(np.float16).copy()
        in_maps.append(m)
    return in_maps


def run(x, W_qkv, b_qkv, W_gate, b_gate, trace=False, **run_kwargs):
    with_bias = bool(np.any(np.asarray(b_qkv)) or np.any(np.asarray(b_gate)))
    nc = _get_nc(with_bias)
    in_maps = _prep_in_maps(x, W_qkv, b_qkv, W_gate, b_gate, with_bias)
    res = run_bass_kernel_spmd(nc, in_maps, list(range(8)), trace=trace, **run_kwargs)
    out = np.empty((B, S, D), dtype=np.float32)
    for h in range(8):
        # res[h]["out"]: [NBLK, P, B, HC] -> out[b, s, h*HC:(h+1)*HC]
        o = np.asarray(res.results[h]["out"]).transpose(2, 0, 1, 3)
        out[:, :, h * HC:(h + 1) * HC] = o.reshape(B, S, HC)
    return out, res


def kernel(x, W_qkv, b_qkv, W_gate, b_gate):
    out, _ = run(x, W_qkv, b_qkv, W_gate, b_gate)
    return out


# revision 34
# speedup vs baseline: 1.2014x; 1.0057x over previous
"""Trainium2 Bass kernel for nn_LinearLatentKernel_84834194031187.

Computes, for x:[B,S,D], W_qkv:[3D,D], W_gate:[D,D] (fp32):
    qkv = x @ W_qkv.T + b_qkv ; q,k,v = split(qkv)
    kv_state = cumsum(k*v, axis=seq)
    out = q * kv_state * sigmoid(x @ W_gate.T + b_gate)

Sharding: 8-way channel split. Core h handles channels [h*128,(h+1)*128) of
q, k, v and the gate for ALL four batches, producing out[:, :, h*128:...].
This keeps each core's weight slice at 1MB (vs 4MB for a batch x half-D
split), which matters because the startup ramp is HBM-bandwidth-bound:
block 0 cannot finish before its weights land.

x is host-pretransposed and pre-tiled into [NBLK, 128, B, KT, 128] fp16 so
each seq block's x^T tiles (all 4 batches) arrive via contiguous DMAs
(8KB/partition) and feed the PE stationary port directly -- no on-device
transposes.

Per seq block of 128 rows (partition dim = seq):
  - one PSUM bank per batch accumulates [k|v|q|g] x 128 channels over the
    8 contraction tiles (fp16 operands, fp32 PSUM, N=512 matmuls). The
    batch-outer matmul order staggers bank completion so each bank is
    drained (kv product, sigmoid, q*g on DVE/ACT) while later batches'
    matmuls still run: all four banks stay single-buffered with no stalls.
  - carry fold: kv_b[0,:] += carry_b (per-batch DVE adds; placed after the
    PSUM drain mid-stream, right after the kv product on the final block);
    a single upper-triangular fp16 matmul (N=512: the four batches' columns
    are independent) then yields the block cumsum INCLUDING the carry, and
    its row 127 IS the carry for the next block: a 1-lane scalar copy +
    SBUF-to-SBUF DMA moves it from partition 127 to 0.
  - The PE queue is software-pipelined one block: cumsum matmuls for block
    i-1 are enqueued between batch groups of block i's projections.
  - out = (q * sigmoid(g)) * kv_state, one 256KB DMA per block.

A short burst of dummy "warmup" matmuls (no DMA dependencies) runs first so
the PE's HAM reaches its fast state during the initial DMA ramp.
"""

import numpy as np

import concourse.bacc as bacc
import concourse.tile as tile
import concourse.mybir as mybir
from concourse.bass_utils import run_bass_kernel_spmd

B, S, D = 4, 4096, 1024
HC = 128         # channels per core (D / 8 cores)
W4 = 4 * HC      # k|v|q|g channel block per core = 512
P = 128
NBLK = S // P    # 32 seq blocks
KT = D // P      # 8 contraction tiles

f32 = mybir.dt.float32
f16 = mybir.dt.float16

_NC_CACHE = {}


def _build(with_bias: bool):
    nc = bacc.Bacc("TRN2", target_bir_lowering=False)

    # x^T pre-tiled on host: xh[i, p, b, kt, j] = x[b, i*128+j, kt*128+p]
    xh_d = nc.dram_tensor("xh", [NBLK, P, B, KT, P], f16, kind="ExternalInput")
    # weight columns ordered [k | v | q | g], HC channels each
    wt_d = nc.dram_tensor("wt", [KT, P, W4], f16, kind="ExternalInput")
    tri_d = nc.dram_tensor("tri", [P, P], f16, kind="ExternalInput")
    if with_bias:
        onesrow_d = nc.dram_tensor("onesrow", [1, P], f16, kind="ExternalInput")
        bias_d = nc.dram_tensor("bias", [1, W4], f16, kind="ExternalInput")
    # out[i, j, b, c] = result[b, i*128+j, h*128+c]
    out_d = nc.dram_tensor("out", [NBLK, P, B, HC], f32, kind="ExternalOutput")

    with tile.TileContext(nc) as tc:
        with (
            tc.tile_pool(name="consts", bufs=1) as consts,
            tc.tile_pool(name="xtp", bufs=3) as xtp,
            tc.tile_pool(name="kp", bufs=2) as kp,
            tc.tile_pool(name="gp", bufs=2) as gp,
            tc.tile_pool(name="kvp", bufs=2) as kvp,
            tc.tile_pool(name="qgp", bufs=2) as qgp,
            tc.tile_pool(name="outp", bufs=3) as outp,
            tc.tile_pool(name="tmpp", bufs=2) as tmpp,
            tc.tile_pool(name="carryp", bufs=2) as carryp,
            tc.tile_pool(name="pmm", bufs=1, space="PSUM") as pmm,
            tc.tile_pool(name="pcs_pool", bufs=2, space="PSUM") as pcs_pool,
            tc.tile_pool(name="pwm", bufs=1, space="PSUM") as pwm,
        ):
            warm_a = consts.tile([P, P], f16, tag="warm_a")
            nc.vector.memset(warm_a[:], 0.0)
            warm_b = consts.tile([P, 512], f16, tag="warm_b")
            nc.vector.memset(warm_b[:], 0.0)
            pwarm = pwm.tile([P, 512], f32, tag="pwarm")

            def warm(n):
                for _ in range(n):
                    nc.tensor.matmul(pwarm[:], warm_a[:], warm_b[:],
                                     start=True, stop=True)

            warm(12)

            # block 0/1 inputs split across two trigger queues; weights on
            # gpsimd+scalar so everything rides different DMA rings
            xt0 = xtp.tile([P, B, KT, P], f16, tag="xt", name="xt0")
            nc.sync.dma_start(xt0[:, 0:2], xh_d[0, :, 0:2])
            nc.scalar.dma_start(xt0[:, 2:4], xh_d[0, :, 2:4])
            wt_sb = consts.tile([P, KT, W4], f16, tag="wt")
            for kt in range(KT):
                eng = nc.gpsimd if kt % 2 == 0 else nc.scalar
                eng.dma_start(wt_sb[:, kt, :], wt_d[kt])
            xt1 = xtp.tile([P, B, KT, P], f16, tag="xt", name="xt1")
            nc.sync.dma_start(xt1[:, 0:2], xh_d[1, :, 0:2])
            nc.scalar.dma_start(xt1[:, 2:4], xh_d[1, :, 2:4])

            tri_sb = consts.tile([P, P], f16, tag="tri")
            nc.sync.dma_start(tri_sb[:], tri_d[:])
            if with_bias:
                onesrow_sb = consts.tile([1, P], f16, tag="onesrow")
                nc.sync.dma_start(onesrow_sb[:], onesrow_d[:])
                bias_sb = consts.tile([1, W4], f16, tag="bias")
                nc.sync.dma_start(bias_sb[:], bias_d[:])

            xts = {0: xt0, 1: xt1}
            pending = None      # (kvs, qgs, i) awaiting cumsum+output

            def proj_batch(ps_b, xt, b):
                for kt in range(KT):
                    nc.tensor.matmul(
                        ps_b[:], xt[:, b, kt, :], wt_sb[:, kt, :],
                        start=(kt == 0),
                        stop=(kt == KT - 1 and not with_bias),
                    )
                if with_bias:
                    nc.tensor.matmul(ps_b[:], onesrow_sb[:], bias_sb[:],
                                     start=False, stop=True)

            def cumsum_mms(pend):
                # PE part of block j's cumsum: ONE N=512 tri matmul covers all
                # four batches (columns independent); row 127 = next carry,
                # moved 127->0 via 1-lane copy + tiny SBUF DMA.
                kv_all, qg_all, j = pend
                pcs = pcs_pool.tile([P, B, HC], f32, tag="pcs")
                nc.tensor.matmul(pcs[:], tri_sb[:], kv_all[:],
                                 start=True, stop=True)
                carry_new = None
                if j < NBLK - 1:
                    tmp = tmpp.tile([P, B, HC], f32, tag="tmp")
                    nc.scalar.activation(tmp[96:P], pcs[96:P],
                                         mybir.ActivationFunctionType.Copy)
                    carry_new = carryp.tile([1, B, HC], f32, tag="carry")
                    nc.gpsimd.dma_start(carry_new[0:1], tmp[P - 1:P])
                return pcs, carry_new

            def emit_out(pend, pcs):
                _, qg_all, j = pend
                ob = outp.tile([P, B, HC], f32, tag="ob")
                nc.vector.tensor_mul(out=ob[:], in0=qg_all[:], in1=pcs[:])
                nc.sync.dma_start(out_d[j], ob[:])

            for i in range(NBLK):
                if i + 2 < NBLK:
                    xt = xtp.tile([P, B, KT, P], f16, tag="xt")
                    nc.sync.dma_start(xt[:, 0:2], xh_d[i + 2, :, 0:2])
                    nc.scalar.dma_start(xt[:, 2:4], xh_d[i + 2, :, 2:4])
                    xts[i + 2] = xt
                xt = xts.pop(i)

                ps = [pmm.tile([P, W4], f32, tag=f"ps{b}", name=f"ps{b}")
                      for b in range(B)]
                kv_all = kvp.tile([P, B, HC], f16, tag="kv")
                qg_all = qgp.tile([P, B, HC], f32, tag="qg")

                proj_batch(ps[0], xt, 0)
                proj_batch(ps[1], xt, 1)

                # block i-1's cumsum matmuls, mid-block on the PE queue
                pcs_prev = None
                if pending is not None:
                    pcs_prev, carry_prev = cumsum_mms(pending)

                proj_batch(ps[2], xt, 2)
                proj_batch(ps[3], xt, 3)

                for b in range(B):
                    k_sb = kp.tile([P, HC], f32, tag=f"k{b}")
                    nc.scalar.activation(k_sb[:], ps[b][:, 0:HC],
                                         mybir.ActivationFunctionType.Copy)
                    nc.vector.tensor_mul(out=kv_all[:, b, :], in0=k_sb[:],
                                         in1=ps[b][:, HC:2 * HC])
                    if i == NBLK - 1:
                        # final block: carry arrived long ago; folding right
                        # after the kv product unblocks the tail cumsum
                        nc.vector.tensor_add(out=kv_all[0:1, b, :],
                                             in0=kv_all[0:1, b, :],
                                             in1=carry_prev[0:1, b, :])
                    if b == 0 and pending is not None:
                        emit_out(pending, pcs_prev)
                    g_sb = gp.tile([P, HC], f32, tag=f"g{b}")
                    nc.scalar.activation(g_sb[:], ps[b][:, 3 * HC:4 * HC],
                                         mybir.ActivationFunctionType.Sigmoid)
                    nc.vector.tensor_mul(out=qg_all[:, b, :], in0=g_sb[:],
                                         in1=ps[b][:, 2 * HC:3 * HC])
                    if 0 < i < NBLK - 1:
                        # carry fold: kv[0,:] += carry (cumsum row 127);
                        # after qg so the PSUM drain never waits on the
                        # in-flight carry DMA
                        nc.vector.tensor_add(out=kv_all[0:1, b, :],
                                             in0=kv_all[0:1, b, :],
                                             in1=carry_prev[0:1, b, :])

                pending = (kv_all, qg_all, i)

            # final flush: per-batch cumsum + product + store, so each
            # piece drains as soon as its own kv/fold is ready
            kv_all, qg_all, j = pending
            pcs = pcs_pool.tile([P, B, HC], f32, tag="pcs")
            ob = outp.tile([P, B, HC], f32, tag="ob")
            for b in range(B):
                nc.tensor.matmul(pcs[:, b, :], tri_sb[:], kv_all[:, b, :],
                                 start=True, stop=True)
                nc.vector.tensor_mul(out=ob[:, b, :], in0=qg_all[:, b, :],
                                     in1=pcs[:, b, :])
                nc.sync.dma_start(out_d[j, :, b], ob[:, b, :])

    nc.compile()
    return nc


def _get_nc(with_bias: bool):
    if with_bias not in _NC_CACHE:
        _NC_CACHE[with_bias] = _build(with_bias)
    return _NC_CACHE[with_bias]


def _prep_in_maps(x, W_qkv, b_qkv, W_gate, b_gate, with_bias):
    x = np.asarray(x, dtype=np.float32).astype(np.float16)
    W_qkv = np.asarray(W_qkv, dtype=np.float32)
    W_gate = np.asarray(W_gate, dtype=np.float32)

    consts = {
        "tri": np.triu(np.ones((P, P), dtype=np.float16)),
    }
    if with_bias:
        consts["onesrow"] = np.ones((1, P), dtype=np.float16)

    # xh[i, p, b, kt, j] = x[b, i*128+j, kt*128+p]  (shared by all cores)
    xh = np.ascontiguousarray(
        x.reshape(B, NBLK, P, KT, P).transpose(1, 4, 0, 3, 2))

    in_maps = []
    for h in range(8):
        sl = slice(h * HC, (h + 1) * HC)
        wt = np.concatenate(
            [W_qkv[D + h * HC:D + (h + 1) * HC],        # k rows
             W_qkv[2 * D + h * HC:2 * D + (h + 1) * HC],  # v rows
             W_qkv[sl],                                   # q rows
             W_gate[sl]], axis=0                          # g rows
        ).T.astype(np.float16)                            # [D, 512]
        wt = np.ascontiguousarray(wt.reshape(KT, P, W4))
        m = {"xh": xh, "wt": wt, **consts}
        if with_bias:
            bq = np.asarray(b_qkv, dtype=np.float32)
            bg = np.asarray(b_gate, dtype=np.float32)
            m["bias"] = np.concatenate(
                [bq[D + h * HC:D + (h + 1) * HC],
                 bq[2 * D + h * HC:2 * D + (h + 1) * HC],
                 bq[sl], bg[sl]]
            )[None, :].astype(np.float16).copy()
        in_maps.append(m)
    return in_maps


def run(x, W_qkv, b_qkv, W_gate, b_gate, trace=False, **run_kwargs):
    with_bias = bool(np.any(np.asarray(b_qkv)) or np.any(np.asarray(b_gate)))
    nc = _get_nc(with_bias)
    in_maps = _prep_in_maps(x, W_qkv, b_qkv, W_gate, b_gate, with_bias)
    res = run_bass_kernel_spmd(nc, in_maps, list(range(8)), trace=trace, **run_kwargs)
    out = np.empty((B, S, D), dtype=np.float32)
    for h in range(8):
        # res[h]["out"]: [NBLK, P, B, HC] -> out[b, s, h*HC:(h+1)*HC]
        o = np.asarray(res.results[h]["out"]).transpose(2, 0, 1, 3)
        out[:, :, h * HC:(h + 1) * HC] = o.reshape(B, S, HC)
    return out, res


def kernel(x, W_qkv, b_qkv, W_gate, b_gate):
    out, _ = run(x, W_qkv, b_qkv, W_gate, b_gate)
    return out


# revision 35
# speedup vs baseline: 1.2105x; 1.0076x over previous
"""Trainium2 Bass kernel for nn_LinearLatentKernel_84834194031187.

Computes, for x:[B,S,D], W_qkv:[3D,D], W_gate:[D,D] (fp32):
    qkv = x @ W_qkv.T + b_qkv ; q,k,v = split(qkv)
    kv_state = cumsum(k*v, axis=seq)
    out = q * kv_state * sigmoid(x @ W_gate.T + b_gate)

Sharding: 8-way channel split. Core h handles channels [h*128,(h+1)*128) of
q, k, v and the gate for ALL four batches, producing out[:, :, h*128:...].
This keeps each core's weight slice at 1MB (vs 4MB for a batch x half-D
split), which matters because the startup ramp is HBM-bandwidth-bound:
block 0 cannot finish before its weights land.

x is host-pretransposed and pre-tiled into [NBLK, 128, B, KT, 128] fp16 so
each seq block's x^T tiles (all 4 batches) arrive via contiguous DMAs
(8KB/partition) and feed the PE stationary port directly -- no on-device
transposes.

Per seq block of 128 rows (partition dim = seq):
  - one PSUM bank per batch accumulates [k|v|q|g] x 128 channels over the
    8 contraction tiles (fp16 operands, fp32 PSUM, N=512 matmuls). The
    batch-outer matmul order staggers bank completion so each bank is
    drained (kv product, sigmoid, q*g on DVE/ACT) while later batches'
    matmuls still run: all four banks stay single-buffered with no stalls.
  - carry fold: kv_b[0,:] += carry_b (per-batch DVE adds; placed after the
    PSUM drain mid-stream, right after the kv product on the final block);
    a single upper-triangular fp16 matmul (N=512: the four batches' columns
    are independent) then yields the block cumsum INCLUDING the carry, and
    its row 127 IS the carry for the next block: a 1-lane scalar copy +
    SBUF-to-SBUF DMA moves it from partition 127 to 0.
  - The PE queue is software-pipelined one block: cumsum matmuls for block
    i-1 are enqueued between batch groups of block i's projections.
  - out = (q * sigmoid(g)) * kv_state, one 256KB DMA per block.

A short burst of dummy "warmup" matmuls (no DMA dependencies) runs first so
the PE's HAM reaches its fast state during the initial DMA ramp.
"""

import numpy as np

import concourse.bacc as bacc
import concourse.tile as tile
import concourse.mybir as mybir
from concourse.bass_utils import run_bass_kernel_spmd

B, S, D = 4, 4096, 1024
HC = 128         # channels per core (D / 8 cores)
W4 = 4 * HC      # k|v|q|g channel block per core = 512
P = 128
NBLK = S // P    # 32 seq blocks
KT = D // P      # 8 contraction tiles

f32 = mybir.dt.float32
f16 = mybir.dt.float16

_NC_CACHE = {}


def _build(with_bias: bool):
    nc = bacc.Bacc("TRN2", target_bir_lowering=False)

    # x^T pre-tiled on host: xh[i, p, b, kt, j] = x[b, i*128+j, kt*128+p]
    xh_d = nc.dram_tensor("xh", [NBLK, P, B, KT, P], f16, kind="ExternalInput")
    # weight columns ordered [k | v | q | g], HC channels each
    wt_d = nc.dram_tensor("wt", [KT, P, W4], f16, kind="ExternalInput")
    tri_d = nc.dram_tensor("tri", [P, P], f16, kind="ExternalInput")
    if with_bias:
        onesrow_d = nc.dram_tensor("onesrow", [1, P], f16, kind="ExternalInput")
        bias_d = nc.dram_tensor("bias", [1, W4], f16, kind="ExternalInput")
    # out[i, j, b, c] = result[b, i*128+j, h*128+c]
    out_d = nc.dram_tensor("out", [NBLK, P, B, HC], f32, kind="ExternalOutput")

    with tile.TileContext(nc) as tc:
        with (
            tc.tile_pool(name="consts", bufs=1) as consts,
            tc.tile_pool(name="xtp", bufs=3) as xtp,
            tc.tile_pool(name="kp", bufs=2) as kp,
            tc.tile_pool(name="gp", bufs=2) as gp,
            tc.tile_pool(name="kvp", bufs=2) as kvp,
            tc.tile_pool(name="qgp", bufs=2) as qgp,
            tc.tile_pool(name="outp", bufs=3) as outp,
            tc.tile_pool(name="tmpp", bufs=2) as tmpp,
            tc.tile_pool(name="carryp", bufs=2) as carryp,
            tc.tile_pool(name="pmm", bufs=1, space="PSUM") as pmm,
            tc.tile_pool(name="pcs_pool", bufs=2, space="PSUM") as pcs_pool,
            tc.tile_pool(name="pwm", bufs=1, space="PSUM") as pwm,
        ):
            warm_a = consts.tile([P, P], f16, tag="warm_a")
            nc.vector.memset(warm_a[:], 0.0)
            warm_b = consts.tile([P, 512], f16, tag="warm_b")
            nc.vector.memset(warm_b[:], 0.0)
            pwarm = pwm.tile([P, 512], f32, tag="pwarm")

            def warm(n):
                for _ in range(n):
                    nc.tensor.matmul(pwarm[:], warm_a[:], warm_b[:],
                                     start=True, stop=True)

            warm(12)

            # block 0/1 inputs split across two trigger queues; weights on
            # gpsimd+scalar so everything rides different DMA rings. Ring
            # order matches block 0's consumption: batch 0/1 input quarters
            # and ALL weights (needed within the first 2us of matmuls) go
            # ahead of the batch 2/3 input quarters (needed ~4us in).
            xt0 = xtp.tile([P, B, KT, P], f16, tag="xt", name="xt0")
            nc.sync.dma_start(xt0[:, 0:2], xh_d[0, :, 0:2])
            wt_sb = consts.tile([P, KT, W4], f16, tag="wt")
            for kt in range(KT):
                eng = nc.gpsimd if kt % 2 == 0 else nc.scalar
                eng.dma_start(wt_sb[:, kt, :], wt_d[kt])
            nc.scalar.dma_start(xt0[:, 2:4], xh_d[0, :, 2:4])
            xt1 = xtp.tile([P, B, KT, P], f16, tag="xt", name="xt1")
            nc.sync.dma_start(xt1[:, 0:2], xh_d[1, :, 0:2])
            nc.scalar.dma_start(xt1[:, 2:4], xh_d[1, :, 2:4])

            tri_sb = consts.tile([P, P], f16, tag="tri")
            nc.sync.dma_start(tri_sb[:], tri_d[:])
            if with_bias:
                onesrow_sb = consts.tile([1, P], f16, tag="onesrow")
                nc.sync.dma_start(onesrow_sb[:], onesrow_d[:])
                bias_sb = consts.tile([1, W4], f16, tag="bias")
                nc.sync.dma_start(bias_sb[:], bias_d[:])

            xts = {0: xt0, 1: xt1}
            pending = None      # (kvs, qgs, i) awaiting cumsum+output

            def proj_batch(ps_b, xt, b):
                for kt in range(KT):
                    nc.tensor.matmul(
                        ps_b[:], xt[:, b, kt, :], wt_sb[:, kt, :],
                        start=(kt == 0),
                        stop=(kt == KT - 1 and not with_bias),
                    )
                if with_bias:
                    nc.tensor.matmul(ps_b[:], onesrow_sb[:], bias_sb[:],
                                     start=False, stop=True)

            def cumsum_mms(pend):
                # PE part of block j's cumsum: ONE N=512 tri matmul covers all
                # four batches (columns independent); row 127 = next carry,
                # moved 127->0 via 1-lane copy + tiny SBUF DMA.
                kv_all, qg_all, j = pend
                pcs = pcs_pool.tile([P, B, HC], f32, tag="pcs")
                nc.tensor.matmul(pcs[:], tri_sb[:], kv_all[:],
                                 start=True, stop=True)
                carry_new = None
                if j < NBLK - 1:
                    tmp = tmpp.tile([P, B, HC], f32, tag="tmp")
                    nc.scalar.activation(tmp[96:P], pcs[96:P],
                                         mybir.ActivationFunctionType.Copy)
                    carry_new = carryp.tile([1, B, HC], f32, tag="carry")
                    nc.gpsimd.dma_start(carry_new[0:1], tmp[P - 1:P])
                return pcs, carry_new

            def emit_out(pend, pcs):
                _, qg_all, j = pend
                ob = outp.tile([P, B, HC], f32, tag="ob")
                nc.vector.tensor_mul(out=ob[:], in0=qg_all[:], in1=pcs[:])
                nc.sync.dma_start(out_d[j], ob[:])

            for i in range(NBLK):
                if i + 2 < NBLK:
                    xt = xtp.tile([P, B, KT, P], f16, tag="xt")
                    nc.sync.dma_start(xt[:, 0:2], xh_d[i + 2, :, 0:2])
                    nc.scalar.dma_start(xt[:, 2:4], xh_d[i + 2, :, 2:4])
                    xts[i + 2] = xt
                xt = xts.pop(i)

                ps = [pmm.tile([P, W4], f32, tag=f"ps{b}", name=f"ps{b}")
                      for b in range(B)]
                kv_all = kvp.tile([P, B, HC], f16, tag="kv")
                qg_all = qgp.tile([P, B, HC], f32, tag="qg")

                proj_batch(ps[0], xt, 0)
                proj_batch(ps[1], xt, 1)

                # block i-1's cumsum matmuls, mid-block on the PE queue
                pcs_prev = None
                if pending is not None:
                    pcs_prev, carry_prev = cumsum_mms(pending)

                proj_batch(ps[2], xt, 2)
                proj_batch(ps[3], xt, 3)

                for b in range(B):
                    k_sb = kp.tile([P, HC], f32, tag=f"k{b}")
                    nc.scalar.activation(k_sb[:], ps[b][:, 0:HC],
                                         mybir.ActivationFunctionType.Copy)
                    nc.vector.tensor_mul(out=kv_all[:, b, :], in0=k_sb[:],
                                         in1=ps[b][:, HC:2 * HC])
                    if i == NBLK - 1:
                        # final block: carry arrived long ago; folding right
                        # after the kv product unblocks the tail cumsum
                        nc.vector.tensor_add(out=kv_all[0:1, b, :],
                                             in0=kv_all[0:1, b, :],
                                             in1=carry_prev[0:1, b, :])
                    if b == 0 and pending is not None:
                        emit_out(pending, pcs_prev)
                    g_sb = gp.tile([P, HC], f32, tag=f"g{b}")
                    nc.scalar.activation(g_sb[:], ps[b][:, 3 * HC:4 * HC],
                                         mybir.ActivationFunctionType.Sigmoid)
                    nc.vector.tensor_mul(out=qg_all[:, b, :], in0=g_sb[:],
                                         in1=ps[b][:, 2 * HC:3 * HC])
                    if 0 < i < NBLK - 1:
                        # carry fold: kv[0,:] += carry (cumsum row 127);
                        # after qg so the PSUM drain never waits on the
                        # in-flight carry DMA
                        nc.vector.tensor_add(out=kv_all[0:1, b, :],
                                             in0=kv_all[0:1, b, :],
                                             in1=carry_prev[0:1, b, :])

                pending = (kv_all, qg_all, i)

            # final flush: per-batch cumsum + product + store, so each
            # piece drains as soon as its own kv/fold is ready
            kv_all, qg_all, j = pending
            pcs = pcs_pool.tile([P, B, HC], f32, tag="pcs")
            ob = outp.tile([P, B, HC], f32, tag="ob")
            for b in range(B):
                nc.tensor.matmul(pcs[:, b, :], tri_sb[:], kv_all[:, b, :],
                                 start=True, stop=True)
                nc.vector.tensor_mul(out=ob[:, b, :], in0=qg_all[:, b, :],
                                     in1=pcs[:, b, :])
                nc.sync.dma_start(out_d[j, :, b], ob[:, b, :])

    nc.compile()
    return nc


def _get_nc(with_bias: bool):
    if with_bias not in _NC_CACHE:
        _NC_CACHE[with_bias] = _build(with_bias)
    return _NC_CACHE[with_bias]


def _prep_in_maps(x, W_qkv, b_qkv, W_gate, b_gate, with_bias):
    x = np.asarray(x, dtype=np.float32).astype(np.float16)
    W_qkv = np.asarray(W_qkv, dtype=np.float32)
    W_gate = np.asarray(W_gate, dtype=np.float32)

    consts = {
        "tri": np.triu(np.ones((P, P), dtype=np.float16)),
    }
    if with_bias:
        consts["onesrow"] = np.ones((1, P), dtype=np.float16)

    # xh[i, p, b, kt, j] = x[b, i*128+j, kt*128+p]  (shared by all cores)
    xh = np.ascontiguousarray(
        x.reshape(B, NBLK, P, KT, P).transpose(1, 4, 0, 3, 2))

    in_maps = []
    for h in range(8):
        sl = slice(h * HC, (h + 1) * HC)
        wt = np.concatenate(
            [W_qkv[D + h * HC:D + (h + 1) * HC],        # k rows
             W_qkv[2 * D + h * HC:2 * D + (h + 1) * HC],  # v rows
             W_qkv[sl],                                   # q rows
             W_gate[sl]], axis=0                          # g rows
        ).T.astype(np.float16)                            # [D, 512]
        wt = np.ascontiguousarray(wt.reshape(KT, P, W4))
        m = {"xh": xh, "wt": wt, **consts}
        if with_bias:
            bq = np.asarray(b_qkv, dtype=np.float32)
            bg = np.asarray(b_gate, dtype=np.float32)
            m["bias"] = np.concatenate(
                [bq[D + h * HC:D + (h + 1) * HC],
                 bq[2 * D + h * HC:2 * D + (h + 1) * HC],
                 bq[sl], bg[sl]]
            )[None, :].astype(np.float16).copy()
        in_maps.append(m)
    return in_maps


def run(x, W_qkv, b_qkv, W_gate, b_gate, trace=False, **run_kwargs):
    with_bias = bool(np.any(np.asarray(b_qkv)) or np.any(np.asarray(b_gate)))
    nc = _get_nc(with_bias)
    in_maps = _prep_in_maps(x, W_qkv, b_qkv, W_gate, b_gate, with_bias)
    res = run_bass_kernel_spmd(nc, in_maps, list(range(8)), trace=trace, **run_kwargs)
    out = np.empty((B, S, D), dtype=np.float32)
    for h in range(8):
        # res[h]["out"]: [NBLK, P, B, HC] -> out[b, s, h*HC:(h+1)*HC]
        o = np.asarray(res.results[h]["out"]).transpose(2, 0, 1, 3)
        out[:, :, h * HC:(h + 1) * HC] = o.reshape(B, S, HC)
    return out, res


def kernel(x, W_qkv, b_qkv, W_gate, b_gate):
    out, _ = run(x, W_qkv, b_qkv, W_gate, b_gate)
    return out
